# revision 16
# baseline (speedup 1.0000x reference)
"""Trainium2 Bass/Tile kernel for nn_FB_FMM (sparse_attention).

Computation (per batch element b, with N = H*W = 4096 tokens, C=256, D=32):
  1. Self-attention:  sa_out = attn(conv(x,sa_wq), conv(x,sa_wk), conv(x,sa_wv))
     x' = sa_gamma * sa_out + x
  2. Masked cross-attention (FB_FMM):
     ff = mask * x'; fb = (1-mask) * x'
     sw_bg = attn(conv(ff,wq), conv(fb,wk), conv(fb,wv))
     out = x' + gamma * ff * (std(sw_bg)/std(ff))    [per-channel std, ddof=1]

Mask structure exploited (attention is permutation-equivariant, so the host
permutes tokens background-first per batch and un-permutes the output):
  - fb = 0 at foreground tokens: their layer-2 keys are the constant bk and
    their (bias-free) values are 0, so keys reduce to the first NBGT_pad
    permuted tiles plus a closed-form denominator correction
    n_excl * exp(Q_i . bk).  The layer-2 V bias drops out entirely because
    sum_j A_ij = 1 makes it a per-channel shift and the FMM std is
    shift-invariant.
  - ff = 0 at background tokens: their layer-2 query is exactly bq, so all
    pure-background rows produce ONE shared attention row.  Rows of the
    first TL tiles are skipped; their stats enter via a single extra "bq
    column" appended to Q2 whose accumulated weight is TL*128, split
    exactly across the 4 cores of a group.  Their final output is just x'
    (ff=0), DMA'd straight after layer 1.

Sharding: 8 cores = 2 batch groups x 4-way row sharding.  Global token tile
t (of 32) is dealt to core t%4, so background tiles (= layer-2 keys) and
live rows stay balanced across cores.  Each core's rows are ordered by
global tile index, so its key tiles are a prefix: the AllGather of x' key
columns (bf16) launches right after layer-1 i-chunk 0 and hides under
i-chunk 1.  Gathered key tiles are kept in rank-major order (attention is
order-invariant over keys) so the post-AllGather scatter is one contiguous
DMA per (rank, channel-half).  A tiny [128,8] AllReduce combines the FMM
stats.

Layouts: feature maps channel-major; scores transposed (keys on partitions,
logits small so exp needs no max pass).  The K=32 score matmuls run 2-way
row-tiled (tile_position groups 0/1, K tiles interleaved across partition
groups, Q replicated by widening the conv to M=128 at no cost), halving
score-matmul PE time.  AV keeps V^T slices stationary; the softmax
denominator comes from an M=1 ones-matmul per tile; the reciprocal row is
partition-replicated on the idle GPSIMD engine.  Heavy matmuls in
bf16/float32r (~1e-4 relative rounding).  Layer-1 V bias is folded into the
residual via sum_j A_ij = 1.
"""

import numpy as np

P = 128
G = 32                 # PE row-group granularity
B, C, HH, WW = 2, 256, 64, 64
N = HH * WW            # 4096 tokens
D = 32                 # q/k channels
NCORES = 8
RSH = 4                # row shards per batch group
R = N // RSH           # 1024 query rows per core
NT = N // P            # 32 token tiles
IC = 512               # query i-chunk (one PSUM bank of fp32)
EPS = 1e-5
F32 = np.float32

_CACHE = {}
USE_GPSIMD_BCAST = True


def _chunks(total, cap):
    """Split `total` columns into near-even chunks of width <= cap."""
    n = -(-total // cap)
    base, rem = divmod(total, n)
    out, off = [], 0
    for i in range(n):
        w = base + (1 if i < rem else 0)
        out.append((off, w))
        off += w
    return out


def _build_bass(NBGT_pad, TL):
    """Build the Bass/Tile program (single SPMD NEFF for all 8 cores)."""
    import concourse.bass as bass
    from concourse import bacc, mybir, tile

    f32 = mybir.dt.float32
    f32r = mybir.dt.float32r
    bf16 = mybir.dt.bfloat16
    AX = mybir.AxisListType
    OP = mybir.AluOpType
    AF = mybir.ActivationFunctionType

    KPC = NBGT_pad // 4          # key tiles per core (prefix of its rows)
    PBT = TL // 4                # pure-bg tiles per core (rows skipped in L2)
    LR = R - PBT * P             # live query rows per core
    KC = KPC * P                 # key columns per core
    NT2 = RSH * KPC              # total key tiles (rank-major order)
    BND = (KPC - PBT) * RSH * P  # key cols needing the (1-mask) multiply
    n_excl = N - NBGT_pad * P    # excluded fg keys -> denominator correction
    BQW = TL * 32 - 1            # extra bq-row stat weight per core
    cols2 = LR + 1               # live rows + appended bq column
    CH1 = _chunks(R, IC)
    CH2 = _chunks(cols2, IC)
    CHQ2 = _chunks(LR, IC)

    nc = bacc.Bacc(
        "TRN2", target_bir_lowering=False, debug=False, num_devices=NCORES
    )
    bf16d = mybir.dt.bfloat16

    # ---------------- I/O ----------------
    xf_d = nc.dram_tensor("xf", [C, N], bf16d, kind="ExternalInput")
    xc_d = nc.dram_tensor("xc", [C, R], f32r, kind="ExternalInput")
    mlive_d = nc.dram_tensor("mlive", [1, LR], f32, kind="ExternalInput")
    mbnd_d = nc.dram_tensor("mbnd", [1, max(BND, P)], f32,
                            kind="ExternalInput")
    # q/k weights pre-replicated 4x along the output dim (M=128)
    wqT1_d = nc.dram_tensor("wqT1", [C, P], f32r, kind="ExternalInput")
    wkT1_d = nc.dram_tensor("wkT1", [C, P], bf16d, kind="ExternalInput")
    wvT1_d = nc.dram_tensor("wvT1", [C, C], bf16d, kind="ExternalInput")
    wqT2_d = nc.dram_tensor("wqT2", [C, P], f32r, kind="ExternalInput")
    wkT2_d = nc.dram_tensor("wkT2", [C, P], bf16d, kind="ExternalInput")
    wvT2_d = nc.dram_tensor("wvT2", [C, C], bf16d, kind="ExternalInput")
    # consts columns: 0 sa_gamma, 1 gamma, 2/3 sa_gamma*sa_bv halves,
    # 6 sa_bq, 7 sa_bk, 8 bq, 9 bk (cols 6-9 replicated on all 4 groups)
    consts_d = nc.dram_tensor("consts", [P, 10], f32, kind="ExternalInput")
    out_d = nc.dram_tensor("outc", [C, R], f32, kind="ExternalOutput")

    groups = [[0, 1, 2, 3], [4, 5, 6, 7]]

    with tile.TileContext(nc) as tc:
        from contextlib import ExitStack

        ctx = ExitStack()
        with ctx:
            big = ctx.enter_context(tc.tile_pool(name="big", bufs=1))
            epool = ctx.enter_context(tc.tile_pool(name="epool", bufs=6))
            onpool = ctx.enter_context(tc.tile_pool(name="onpool", bufs=3))
            sqpool = ctx.enter_context(tc.tile_pool(name="sqpool", bufs=2))
            fbpool = ctx.enter_context(tc.tile_pool(name="fbpool", bufs=4))
            rcpool = ctx.enter_context(tc.tile_pool(name="rcpool", bufs=4))
            finpool = ctx.enter_context(tc.tile_pool(name="finpool", bufs=2))
            misc = ctx.enter_context(tc.tile_pool(name="misc", bufs=1))
            psA = ctx.enter_context(
                tc.tile_pool(name="psA", bufs=2, space="PSUM")
            )
            psS = ctx.enter_context(
                tc.tile_pool(name="psS", bufs=4, space="PSUM")
            )
            psO = ctx.enter_context(
                tc.tile_pool(name="psO", bufs=2, space="PSUM")
            )
            dram = ctx.enter_context(
                tc.tile_pool(name="dram", bufs=1, space="DRAM")
            )

            # ------------- persistent SBUF tiles -------------
            xc_sb = big.tile([P, 2, R], f32r, tag="xc", name="xc_sb")
            mlive_sb = big.tile([P, LR], f32, tag="mlive", name="mlive_sb")
            mbnd_sb = big.tile([P, max(BND, P)], f32, tag="mbnd",
                               name="mbnd_sb")
            xp_sb = big.tile([P, 2, R], f32, tag="xp", name="xp_sb")
            xp16_sb = big.tile([P, 2, KC], bf16, tag="xp16", name="xp16_sb")
            ff_sb = big.tile([P, 2, LR], f32r, tag="ff", name="ff_sb")
            wqT1_sb = big.tile([P, 2, P], f32r, tag="wqT1", name="wqT1_sb")
            wkT1_sb = big.tile([P, 2, P], bf16, tag="wkT1", name="wkT1_sb")
            wvT1_sb = big.tile([P, 2, C], bf16, tag="wvT1", name="wvT1_sb")
            wqT2_sb = big.tile([P, 2, P], f32r, tag="wqT2", name="wqT2_sb")
            wkT2_sb = big.tile([P, 2, P], bf16, tag="wkT2", name="wkT2_sb")
            wvT2_sb = big.tile([P, 2, C], bf16, tag="wvT2", name="wvT2_sb")
            consts_sb = big.tile([P, 10], f32, tag="consts", name="consts_sb")
            onesc_sb = big.tile([P, 1], bf16, tag="onesc", name="onesc_sb")
            onesr_sb = big.tile([1, P], f32r, tag="onesr", name="onesr_sb")
            bk16_sb = big.tile([D, 1], bf16, tag="bk16", name="bk16_sb")
            stats_sb = misc.tile([P, 8], f32, tag="stats", name="stats_sb")

            # ---- input DMAs: small/early tensors first so the layer-1
            # convs can start while the bulk of x streams in.
            nc.sync.dma_start(out=consts_sb[:], in_=consts_d[:])
            for k in range(2):
                cs = slice(k * P, (k + 1) * P)
                nc.sync.dma_start(out=wqT1_sb[:, k, :], in_=wqT1_d[cs, :])
                nc.sync.dma_start(out=wkT1_sb[:, k, :], in_=wkT1_d[cs, :])
                nc.sync.dma_start(out=wvT1_sb[:, k, :], in_=wvT1_d[cs, :])
            for h in range(2):
                hs = slice(h * IC, (h + 1) * IC)
                for k in range(2):
                    nc.sync.dma_start(
                        out=xc_sb[:, k, hs],
                        in_=xc_d[k * P : (k + 1) * P, hs],
                    )
            xf_sb = big.tile([P, 2, N], bf16, tag="xbig", name="xf_sb")
            for jc in range(4):
                js = slice(jc * (N // 4), (jc + 1) * (N // 4))
                for k in range(2):
                    nc.sync.dma_start(
                        out=xf_sb[:, k, js],
                        in_=xf_d[k * P : (k + 1) * P, js],
                    )
            for k in range(2):
                cs = slice(k * P, (k + 1) * P)
                nc.sync.dma_start(out=wqT2_sb[:, k, :], in_=wqT2_d[cs, :])
                nc.sync.dma_start(out=wkT2_sb[:, k, :], in_=wkT2_d[cs, :])
                nc.sync.dma_start(out=wvT2_sb[:, k, :], in_=wvT2_d[cs, :])
            nc.sync.dma_start(
                out=mlive_sb[:], in_=mlive_d[0, :].partition_broadcast(P)
            )
            nc.sync.dma_start(
                out=mbnd_sb[:], in_=mbnd_d[0, :].partition_broadcast(P)
            )
            nc.vector.memset(onesc_sb[:], 1.0)
            nc.vector.memset(onesr_sb[:].bitcast(f32), 1.0)
            nc.vector.tensor_copy(bk16_sb[:], consts_sb[0:D, 9:10])

            def conv_q(wT4_sb, bias_col, src_of, chlist, out_sb):
                """out (128 x width) = [wT tiled 4x].T @ src + bias: the
                D=32 query rows replicated on all four partition groups."""
                for off, w in chlist:
                    js = slice(off, off + w)
                    ps = psA.tile([P, IC], f32, tag="a", name="q_ps")
                    nc.tensor.matmul(
                        ps[:, 0:w], wT4_sb[:, 0, :], src_of(0, js),
                        start=True, stop=False,
                    )
                    nc.tensor.matmul(
                        ps[:, 0:w], wT4_sb[:, 1, :], src_of(1, js),
                        start=False, stop=True,
                    )
                    nc.vector.tensor_scalar_add(
                        out_sb[:, js], ps[:, 0:w],
                        consts_sb[:, bias_col : bias_col + 1],
                    )

            def conv_k(wT4_sb, bias_col, src0, src1, t0, ntl, k4_sb):
                """K conv for `ntl` consecutive key tiles starting at linear
                tile t0; tile t lands on partition group t%2, slot t//2."""
                ps = psA.tile([P, IC], f32, tag="a", name="k_ps")
                wct = ntl * P
                nc.tensor.matmul(
                    ps[:, 0:wct], wT4_sb[:, 0, :], src0,
                    start=True, stop=False,
                )
                nc.tensor.matmul(
                    ps[:, 0:wct], wT4_sb[:, 1, :], src1,
                    start=False, stop=True,
                )
                for i in range(ntl):
                    t = t0 + i
                    gs = slice(G * (t % 2), G * (t % 2) + G)
                    nc.vector.tensor_scalar_add(
                        k4_sb[gs, t // 2, :],
                        ps[gs, i * P : (i + 1) * P],
                        consts_sb[gs, bias_col : bias_col + 1],
                    )

            def conv_vT(wvT_sb, lhsT0, lhsT1, v_sb, t):
                """v_sb[:, t, :] = (src^T @ wvT) for key tile t (j on
                partitions, channels free)."""
                ps = psA.tile([P, C], f32, tag="a", name="v_ps")
                nc.tensor.matmul(
                    ps[:], lhsT0, wvT_sb[:, 0, :], start=True, stop=False
                )
                nc.tensor.matmul(
                    ps[:], lhsT1, wvT_sb[:, 1, :], start=False, stop=True
                )
                nc.scalar.copy(v_sb[:, t, :], ps[:])

            def attention(q_sb, k4_sb, v_sb, nt, chlist, den_fix, epilogue):
                """Row-chunk attention.  Per i-chunk: S^T = K-tile^T Q with
                score matmuls issued as 2-way row-tiled pairs (concurrent in
                the PE), E = exp(S^T), then O(c,i) accumulates with V^T
                slices stationary and E moving; the denominator row comes
                from an M=1 ones matmul, corrected by den_fix and
                replicated across partitions on GPSIMD."""
                LOOKP = 2
                npair = nt // 2
                for ich, (off, w) in enumerate(chlist):
                    is_ = slice(off, off + w)
                    accs = [
                        psO.tile([P, IC], f32, tag="o", name="acc")
                        for _ in range(2)
                    ]
                    den = psA.tile([1, IC], f32, tag="a", name="den")
                    es = {}

                    def s_exp(t, is_=is_, w=w):
                        g = t % 2
                        gs = slice(G * g, G * g + G)
                        sps = psS.tile([P, IC], f32, tag="s", name="s_ps")
                        nc.tensor.matmul(
                            sps[:, 0:w],
                            k4_sb[gs, t // 2, :],
                            q_sb[gs, is_],
                            start=True, stop=True,
                            tile_position=(G * g, 0),
                        )
                        e_sb = epool.tile([P, IC], bf16, tag="e", name="e_sb")
                        nc.scalar.activation(e_sb[:, 0:w], sps[:, 0:w], AF.Exp)
                        es[t] = e_sb

                    def s_pair(pp):
                        s_exp(2 * pp)
                        s_exp(2 * pp + 1)

                    for pp in range(min(LOOKP, npair)):
                        s_pair(pp)
                    for pp in range(npair):
                        if pp + LOOKP < npair:
                            s_pair(pp + LOOKP)
                        for t in (2 * pp, 2 * pp + 1):
                            e_sb = es.pop(t)
                            for ct in range(2):
                                nc.tensor.matmul(
                                    accs[ct][:, 0:w],
                                    v_sb[:, t, ct * P : (ct + 1) * P],
                                    e_sb[:, 0:w],
                                    start=(t == 0), stop=(t == nt - 1),
                                )
                            nc.tensor.matmul(
                                den[:, 0:w], onesc_sb[:], e_sb[:, 0:w],
                                start=(t == 0), stop=(t == nt - 1),
                            )
                    rrow = rcpool.tile([1, IC], f32, tag="rc", name="rrow")
                    den_fix(ich, off, w, den, rrow)
                    rrep = onpool.tile([P, IC], f32, tag="rr", name="rrep")
                    if USE_GPSIMD_BCAST:
                        nc.gpsimd.partition_broadcast(
                            rrep[:, 0:w], rrow[:, 0:w], channels=P
                        )
                    else:
                        rrow_r = rcpool.tile(
                            [1, IC], f32r, tag="rcr", name="rrow_r"
                        )
                        nc.vector.tensor_copy(rrow_r[:, 0:w], rrow[:, 0:w])
                        rrep_ps = psA.tile(
                            [P, IC], f32, tag="a", name="rrep_ps"
                        )
                        nc.tensor.matmul(
                            rrep_ps[:, 0:w], onesr_sb[:], rrow_r[:, 0:w],
                            start=True, stop=True,
                        )
                        nc.scalar.copy(rrep[:, 0:w], rrep_ps[:, 0:w])
                    epilogue(ich, off, w, accs, rrep)

            # ================= Layer 1: self-attention =================
            q1_sb = big.tile([P, R], bf16, tag="q", name="q1_sb")
            k1_sb = big.tile([P, NT // 2, P], bf16, tag="k", name="k1_sb")
            v1_sb = big.tile([P, NT, C], bf16, tag="v", name="v1_sb")

            conv_q(wqT1_sb, 6, lambda k, js: xc_sb[:, k, js], CH1, q1_sb)
            for jc in range(4):
                for h in range(2):
                    t0 = jc * 8 + h * 4
                    js = slice(t0 * P, (t0 + 4) * P)
                    conv_k(
                        wkT1_sb, 7,
                        xf_sb[:, 0, js], xf_sb[:, 1, js], t0, 4, k1_sb,
                    )
                for tsub in range(8):
                    t = jc * 8 + tsub
                    ts_ = slice(t * P, (t + 1) * P)
                    conv_vT(
                        wvT1_sb, xf_sb[:, 0, ts_], xf_sb[:, 1, ts_],
                        v1_sb, t,
                    )

            def den_fix1(ich, off, w, den, rrow):
                # ~18-bit reciprocal, 5x faster than full precision; den is
                # a positive softmax sum so the edge cases can't occur
                nc.vector.reciprocal_approx_fast(rrow[:, 0:w], den[:, 0:w])

            def epilogue1(ich, off, w, accs, rrep):
                io = slice(off, off + w)
                for ct in range(2):
                    # x' = sa_gamma * (O/den) + sa_gamma*bv + x, fused as
                    # ((O * sa_gamma) * rrep), then ((t + sgb) + x)
                    nc.vector.scalar_tensor_tensor(
                        xp_sb[:, ct, io], accs[ct][:, 0:w],
                        consts_sb[:, 0:1], rrep[:, 0:w],
                        op0=OP.mult, op1=OP.mult,
                    )
                    nc.vector.scalar_tensor_tensor(
                        xp_sb[:, ct, io], xp_sb[:, ct, io],
                        consts_sb[:, 2 + ct : 3 + ct],
                        xc_sb[:, ct, io].bitcast(f32),
                        op0=OP.add, op1=OP.add,
                    )
                    # bf16 cast only for the key-column prefix (AllGather)
                    lo, hi = max(off, 0), min(off + w, KC)
                    if lo < hi:
                        nc.scalar.copy(
                            xp16_sb[:, ct, lo:hi], xp_sb[:, ct, lo:hi]
                        )
                    # pure-bg rows: out = x' directly (ff = 0 there)
                    if off < PBT * P:
                        oe = min(off + w, PBT * P)
                        nc.sync.dma_start(
                            out=out_d[ct * P : (ct + 1) * P, off:oe],
                            in_=xp_sb[:, ct, off:oe],
                        )

            attention(q1_sb, k1_sb, v1_sb, NT, CH1, den_fix1, epilogue1)

            # ====== AllGather x' key columns within each batch group ======
            # Phase A: key cols [0, 512) (ready after i-chunk 0, overlaps
            # i-chunk 1).  Phase B (only if KPC == 5): cols [512, 640).
            # xbg keeps rank-major key-tile order [ct][rank][k][col]; the
            # scatter is one contiguous DMA per (rank, half, phase).
            xbg_sb = big.tile(
                [P, 2, RSH, KPC, P], bf16, tag="xbg", name="xbg_sb"
            )
            phases = [(0, min(KC, IC))]
            if KC > IC:
                phases.append((IC, KC - IC))
            for ph, (poff, pw) in enumerate(phases):
                ag_in = dram.tile(
                    [C, pw], bf16, tag=f"ag_in{ph}", name=f"ag_in{ph}"
                )
                ag_out = dram.tile(
                    [RSH, C, pw], bf16, tag=f"ag_out{ph}", name=f"ag_out{ph}"
                )
                for ct in range(2):
                    nc.sync.dma_start(
                        out=ag_in[ct * P : (ct + 1) * P, :],
                        in_=xp16_sb[:, ct, poff : poff + pw],
                    )
                nc.gpsimd.collective_compute(
                    "AllGather",
                    OP.bypass,
                    replica_groups=groups,
                    ins=[ag_in[:].opt()],
                    outs=[ag_out[:].opt()],
                )
                k0, nk = poff // P, pw // P
                for r in range(RSH):
                    for ct in range(2):
                        nc.sync.dma_start(
                            out=xbg_sb[:, ct, r, k0 : k0 + nk, :],
                            in_=ag_out[r, ct * P : (ct + 1) * P, :],
                        )

            # ============== Layer 2: masked cross-attention ==============
            # feature_f on live rows + its per-channel stats (cols 0-3)
            ffsq = misc.tile([P, LR], f32, tag="ffsq", name="ffsq")
            for ct in range(2):
                nc.vector.tensor_mul(
                    ff_sb[:, ct, :], mlive_sb[:],
                    xp_sb[:, ct, PBT * P : R].bitcast(f32),
                )
                nc.vector.tensor_reduce(
                    stats_sb[:, ct : ct + 1], ff_sb[:, ct, :].bitcast(f32),
                    axis=AX.X, op=OP.add,
                )
                nc.vector.tensor_mul(
                    ffsq[:],
                    ff_sb[:, ct, :].bitcast(f32),
                    ff_sb[:, ct, :].bitcast(f32),
                )
                nc.vector.tensor_reduce(
                    stats_sb[:, 2 + ct : 3 + ct], ffsq[:],
                    axis=AX.X, op=OP.add,
                )

            q2_sb = big.tile([P, cols2], bf16, tag="q2", name="q2_sb")
            conv_q(wqT2_sb, 8, lambda k, js: ff_sb[:, k, js], CHQ2, q2_sb)
            # appended bq column: the shared pure-background query row
            nc.vector.tensor_copy(q2_sb[:, LR : LR + 1], consts_sb[:, 8:9])

            # K2 / V2^T convs over the gathered key columns (rank-major
            # linear tile t = r*KPC + k).  Pure-bg tiles (k < PBT) use x'
            # directly (fb = x'); tiles k >= PBT get the (1-mask) multiply
            # (covers boundary/padding fg columns).
            k2_sb = big.tile([P, NT2 // 2, P], bf16, tag="k2", name="k2_sb")
            v2_sb = big.tile([P, NT2, C], bf16, tag="v2", name="v2_sb")
            for r in range(RSH):
                subs = [(0, PBT), (PBT, KPC - PBT)]
                for k0, nk in subs:
                    if nk <= 0:
                        continue
                    t0 = r * KPC + k0
                    if k0 == 0:
                        srcs = [xbg_sb[:, ct, r, k0 : k0 + nk, :]
                                for ct in range(2)]
                        tile_of = lambda ct, i, r=r, k0=k0: (
                            xbg_sb[:, ct, r, k0 + i, :]
                        )
                    else:
                        boff = r * (KPC - PBT) * P
                        mb3 = mbnd_sb[:, boff : boff + nk * P].rearrange(
                            "p (a b) -> p a b", a=nk
                        )
                        fbt = [
                            fbpool.tile([P, KPC - PBT, P], bf16, tag="fb",
                                        name=f"fb{ct}")
                            for ct in range(2)
                        ]
                        for ct in range(2):
                            nc.vector.tensor_mul(
                                fbt[ct][:, 0:nk, :], mb3,
                                xbg_sb[:, ct, r, k0 : k0 + nk, :],
                            )
                            nc.vector.tensor_sub(
                                fbt[ct][:, 0:nk, :],
                                xbg_sb[:, ct, r, k0 : k0 + nk, :],
                                fbt[ct][:, 0:nk, :],
                            )
                        srcs = [fbt[0][:, 0:nk, :], fbt[1][:, 0:nk, :]]
                        tile_of = lambda ct, i, fbt=fbt: fbt[ct][:, i, :]
                    conv_k(wkT2_sb, 9, srcs[0], srcs[1], t0, nk, k2_sb)
                    for i in range(nk):
                        conv_vT(
                            wvT2_sb, tile_of(0, i), tile_of(1, i),
                            v2_sb, t0 + i,
                        )

            def den_fix2(ich, off, w, den, rrow):
                # den += n_excl * exp(Q2_i . bk)  [excluded fg keys]
                qbk = psA.tile([1, IC], f32, tag="a", name="qbk")
                nc.tensor.matmul(
                    qbk[:, 0:w], bk16_sb[:], q2_sb[0:D, off : off + w],
                    start=True, stop=True,
                )
                eqbk = rcpool.tile([1, IC], f32, tag="eq", name="eqbk")
                nc.scalar.activation(eqbk[:, 0:w], qbk[:, 0:w], AF.Exp)
                dtot = rcpool.tile([1, IC], f32, tag="dt", name="dtot")
                nc.vector.scalar_tensor_tensor(
                    dtot[:, 0:w], eqbk[:, 0:w], float(n_excl), den[:, 0:w],
                    op0=OP.mult, op1=OP.add,
                )
                nc.vector.reciprocal_approx_fast(rrow[:, 0:w], dtot[:, 0:w])

            def epilogue2(ich, off, w, accs, rrep):
                # normalized sw_bg chunk; accumulate per-channel sum/sumsq
                # into stats cols 4-7; add the bq-column extra weight on
                # the chunk holding column LR.
                for ct in range(2):
                    onb = onpool.tile([P, IC], f32, tag="on", name="on2")
                    s1 = rcpool.tile([P, 1], f32, tag="s1", name="s1")
                    nc.vector.scalar_tensor_tensor(
                        onb[:, 0:w], accs[ct][:, 0:w], 1.0, rrep[:, 0:w],
                        op0=OP.mult, op1=OP.mult, accum_out=s1[:],
                    )
                    sqb = sqpool.tile([P, IC], f32, tag="sq", name="sq2")
                    s2 = rcpool.tile([P, 1], f32, tag="s2", name="s2")
                    # square+accumulate on the (otherwise idle) scalar
                    # engine so it runs parallel to the VE stats chain
                    nc.scalar.activation(
                        sqb[:, 0:w], onb[:, 0:w], AF.Square, accum_out=s2[:]
                    )
                    if ich == 0:
                        nc.vector.tensor_copy(
                            stats_sb[:, 4 + ct : 5 + ct], s1[:]
                        )
                        nc.vector.tensor_copy(
                            stats_sb[:, 6 + ct : 7 + ct], s2[:]
                        )
                    else:
                        nc.vector.tensor_add(
                            stats_sb[:, 4 + ct : 5 + ct],
                            stats_sb[:, 4 + ct : 5 + ct], s1[:],
                        )
                        nc.vector.tensor_add(
                            stats_sb[:, 6 + ct : 7 + ct],
                            stats_sb[:, 6 + ct : 7 + ct], s2[:],
                        )
                    if off <= LR < off + w:
                        bq = LR - off
                        nc.vector.scalar_tensor_tensor(
                            stats_sb[:, 4 + ct : 5 + ct],
                            onb[:, bq : bq + 1], float(BQW),
                            stats_sb[:, 4 + ct : 5 + ct],
                            op0=OP.mult, op1=OP.add,
                        )
                        nc.vector.scalar_tensor_tensor(
                            stats_sb[:, 6 + ct : 7 + ct],
                            sqb[:, bq : bq + 1], float(BQW),
                            stats_sb[:, 6 + ct : 7 + ct],
                            op0=OP.mult, op1=OP.add,
                        )

            attention(q2_sb, k2_sb, v2_sb, NT2, CH2, den_fix2, epilogue2)

            # ================== stats AllReduce + FMM ==================
            ar_in = dram.tile([P, 8], f32, tag="ar_in", name="ar_in")
            ar_out = dram.tile([P, 8], f32, tag="ar_out", name="ar_out")
            nc.sync.dma_start(out=ar_in[:], in_=stats_sb[:])
            nc.gpsimd.collective_compute(
                "AllReduce",
                OP.add,
                replica_groups=groups,
                ins=[ar_in[:].opt()],
                outs=[ar_out[:].opt()],
            )
            rst = misc.tile([P, 8], f32, tag="rst", name="rst")
            nc.sync.dma_start(out=rst[:], in_=ar_out[:])

            # var = (S2 - S1^2/N)/(N-1) + EPS (both channel-halves at once)
            varf = misc.tile([P, 2], f32, tag="varf", name="varf")
            varg = misc.tile([P, 2], f32, tag="varg", name="varg")
            ratio = misc.tile([P, 2], f32, tag="ratio", name="ratio")
            for var, s1s, s2s in ((varf, 0, 2), (varg, 4, 6)):
                nc.vector.tensor_mul(
                    var[:], rst[:, s1s : s1s + 2], rst[:, s1s : s1s + 2]
                )
                nc.vector.tensor_scalar(
                    var[:], var[:], -1.0 / N, None, op0=OP.mult
                )
                nc.vector.tensor_add(var[:], var[:], rst[:, s2s : s2s + 2])
                nc.vector.tensor_scalar(
                    var[:], var[:], 1.0 / (N - 1), EPS, op0=OP.mult, op1=OP.add
                )
            nc.vector.reciprocal(varf[:], varf[:])
            nc.vector.tensor_mul(varg[:], varg[:], varf[:])
            nc.scalar.activation(ratio[:], varg[:], AF.Sqrt)
            # fold in gamma
            nc.vector.tensor_scalar_mul(ratio[:], ratio[:], consts_sb[:, 1:2])

            # out = x' + (gamma * std_bg/std_f) * ff on live rows
            for ct in range(2):
                fin = finpool.tile([P, LR], f32, tag="fin", name="fin")
                nc.vector.scalar_tensor_tensor(
                    fin[:], ff_sb[:, ct, :].bitcast(f32),
                    ratio[:, ct : ct + 1], xp_sb[:, ct, PBT * P : R],
                    op0=OP.mult, op1=OP.add,
                )
                nc.sync.dma_start(
                    out=out_d[ct * P : (ct + 1) * P, PBT * P : R], in_=fin[:]
                )

    nc.compile()
    return nc


def _plan(mask):
    """Host-side permutation plan from the mask (per batch)."""
    mflat = [np.asarray(mask[b, 0], dtype=F32).ravel() for b in range(B)]
    perms, nbgs = [], []
    for mb in mflat:
        bg = np.flatnonzero(mb == 0.0)
        fg = np.flatnonzero(mb != 0.0)
        perms.append(np.concatenate([bg, fg]).astype(np.int64))
        nbgs.append(len(bg))
    NBGT_pad = 4 * (-(-max(-(-n // P) for n in nbgs) // 4))
    TL = 4 * ((min(n // P for n in nbgs)) // 4)
    return mflat, perms, nbgs, NBGT_pad, TL


def _prep_inputs(plan, x, mask, sa_wq, sa_bq, sa_wk, sa_bk, sa_wv, sa_bv,
                 sa_gamma, wq, bq, wk, bk, wv, bv, gamma):
    """Build the per-core input maps (host-side sharding + weight layout)."""
    mflat, perms, nbgs, NBGT_pad, TL = plan
    KPC = NBGT_pad // 4
    PBT = TL // 4
    LR = R - PBT * P
    BND = (KPC - PBT) * RSH * P
    x = np.ascontiguousarray(x, dtype=F32)

    import ml_dtypes

    BF16 = ml_dtypes.bfloat16
    wqT1 = np.ascontiguousarray(np.tile(sa_wq.T, (1, 4)), dtype=F32)
    wkT1 = np.ascontiguousarray(np.tile(sa_wk.T, (1, 4)).astype(BF16))
    wvT1 = np.ascontiguousarray(sa_wv.T.astype(BF16))
    wqT2 = np.ascontiguousarray(np.tile(wq.T, (1, 4)), dtype=F32)
    wkT2 = np.ascontiguousarray(np.tile(wk.T, (1, 4)).astype(BF16))
    wvT2 = np.ascontiguousarray(wv.T.astype(BF16))

    consts = np.zeros((P, 10), dtype=F32)
    consts[:, 0] = sa_gamma[0]
    consts[:, 1] = gamma[0]
    sgb = (sa_gamma[0] * sa_bv).astype(F32)
    consts[:, 2] = sgb[0:P]
    consts[:, 3] = sgb[P:C]
    for g in range(4):
        gs = slice(g * D, (g + 1) * D)
        consts[gs, 6] = sa_bq
        consts[gs, 7] = sa_bk
        consts[gs, 8] = bq
        consts[gs, 9] = bk

    in_maps = []
    for g in range(NCORES):
        b, r = g // RSH, g % RSH
        pb = perms[b]
        mb = mflat[b]
        xb = x[b].reshape(C, N)
        xfp = np.ascontiguousarray(xb[:, pb].astype(BF16))
        gtiles = list(range(r, NT, RSH))
        cols = np.concatenate(
            [pb[t * P : (t + 1) * P] for t in gtiles]
        )
        # boundary mask, rank-major: for rank rr, tiles k in [PBT, KPC)
        # correspond to global tiles 4k+rr
        mbnd = np.zeros((1, max(BND, P)), dtype=F32)
        pos = 0
        for rr in range(RSH):
            for k in range(PBT, KPC):
                gt = 4 * k + rr
                mbnd[0, pos : pos + P] = mb[pb[gt * P : (gt + 1) * P]]
                pos += P
        in_maps.append({
            "xf": xfp,
            "xc": np.ascontiguousarray(xb[:, cols]),
            "mlive": np.ascontiguousarray(
                mb[cols[PBT * P :]].reshape(1, LR)
            ),
            "mbnd": mbnd,
            "wqT1": wqT1, "wkT1": wkT1, "wvT1": wvT1,
            "wqT2": wqT2, "wkT2": wkT2, "wvT2": wvT2,
            "consts": consts,
        })
    return in_maps


def kernel(**inputs):
    from concourse import bass_utils

    plan = _plan(np.asarray(inputs["mask"]))
    _, perms, _, NBGT_pad, TL = plan
    key = (NBGT_pad, TL)
    if _CACHE.get("key") != key:
        _CACHE["nc"] = _build_bass(NBGT_pad, TL)
        _CACHE["key"] = key
    nc = _CACHE["nc"]

    in_maps = _prep_inputs(plan, **inputs)
    res = bass_utils.run_bass_kernel_spmd(
        nc, in_maps, core_ids=list(range(NCORES))
    )
    _CACHE["last_results"] = res

    out = np.empty((B, C, N), dtype=F32)
    for g in range(NCORES):
        b, r = g // RSH, g % RSH
        pb = perms[g // RSH]
        oc = res.results[g]["outc"]
        for k, t in enumerate(range(r, NT, RSH)):
            out[b][:, pb[t * P : (t + 1) * P]] = oc[:, k * P : (k + 1) * P]
    return out.reshape(B, C, HH, WW)


# revision 18
# speedup vs baseline: 1.0849x; 1.0849x over previous
"""Trainium2 Bass/Tile kernel for nn_FB_FMM (sparse_attention).

Computation (per batch element b, with N = H*W = 4096 tokens, C=256, D=32):
  1. Self-attention:  sa_out = attn(conv(x,sa_wq), conv(x,sa_wk), conv(x,sa_wv))
     x' = sa_gamma * sa_out + x
  2. Masked cross-attention (FB_FMM):
     ff = mask * x'; fb = (1-mask) * x'
     sw_bg = attn(conv(ff,wq), conv(fb,wk), conv(fb,wv))
     out = x' + gamma * ff * (std(sw_bg)/std(ff))    [per-channel std, ddof=1]

Mask structure exploited (attention is permutation-equivariant, so the host
permutes tokens background-first per batch and un-permutes the output):
  - fb = 0 at foreground tokens: their layer-2 keys are the constant bk and
    their (bias-free) values are 0, so keys reduce to the first NBGT_pad
    permuted tiles plus a closed-form denominator correction
    n_excl * exp(Q_i . bk).  The layer-2 V bias drops out entirely because
    sum_j A_ij = 1 makes it a per-channel shift and the FMM std is
    shift-invariant.
  - ff = 0 at background tokens: their layer-2 query is exactly bq, so all
    pure-background rows produce ONE shared attention row.  Rows of the
    first TL tiles are skipped; their stats enter via a single extra "bq
    column" appended to Q2 whose accumulated weight is TL*128, split
    exactly across the 4 cores of a group.  Their final output is just x'
    (ff=0), DMA'd straight after layer 1.

Sharding: 8 cores = 2 batch groups x 4-way row sharding.  Global token tile
t (of 32) is dealt to core t%4, so background tiles (= layer-2 keys) and
live rows stay balanced across cores.  Each core's rows are ordered by
global tile index, so its key tiles are a prefix: the AllGather of x' key
columns (bf16) launches right after layer-1 i-chunk 0 and hides under
i-chunk 1.  Gathered key tiles are kept in rank-major order (attention is
order-invariant over keys) so the post-AllGather scatter is one contiguous
DMA per (rank, channel-half).  A tiny [128,8] AllReduce combines the FMM
stats.

Layouts: feature maps channel-major; scores transposed (keys on partitions,
logits small so exp needs no max pass).  The K=32 score matmuls run 2-way
row-tiled (tile_position groups 0/1, K tiles interleaved across partition
groups, Q replicated by widening the conv to M=128 at no cost), halving
score-matmul PE time.  AV keeps V^T slices stationary; the softmax
denominator comes from an M=1 ones-matmul per tile; the reciprocal row is
partition-replicated on the idle GPSIMD engine.  Heavy matmuls in
bf16/float32r (~1e-4 relative rounding).  Layer-1 V bias is folded into the
residual via sum_j A_ij = 1.
"""

import numpy as np

P = 128
G = 32                 # PE row-group granularity
B, C, HH, WW = 2, 256, 64, 64
N = HH * WW            # 4096 tokens
D = 32                 # q/k channels
NCORES = 8
RSH = 4                # row shards per batch group
R = N // RSH           # 1024 query rows per core
NT = N // P            # 32 token tiles
IC = 512               # query i-chunk (one PSUM bank of fp32)
EPS = 1e-5
F32 = np.float32

_CACHE = {}
USE_GPSIMD_BCAST = True


def _chunks(total, cap):
    """Split `total` columns into near-even chunks of width <= cap."""
    n = -(-total // cap)
    base, rem = divmod(total, n)
    out, off = [], 0
    for i in range(n):
        w = base + (1 if i < rem else 0)
        out.append((off, w))
        off += w
    return out


def _build_bass(NBGT_pad, TL):
    """Build the Bass/Tile program (single SPMD NEFF for all 8 cores)."""
    import concourse.bass as bass
    from concourse import bacc, mybir, tile

    f32 = mybir.dt.float32
    f32r = mybir.dt.float32r
    bf16 = mybir.dt.bfloat16
    AX = mybir.AxisListType
    OP = mybir.AluOpType
    AF = mybir.ActivationFunctionType

    KPC = NBGT_pad // 4          # key tiles per core (prefix of its rows)
    PBT = TL // 4                # pure-bg tiles per core (rows skipped in L2)
    LR = R - PBT * P             # live query rows per core
    KC = KPC * P                 # key columns per core
    NT2 = RSH * KPC              # total key tiles (rank-major order)
    BND = (KPC - PBT) * RSH * P  # key cols needing the (1-mask) multiply
    n_excl = N - NBGT_pad * P    # excluded fg keys -> denominator correction
    BQW = TL * 32 - 1            # extra bq-row stat weight per core
    cols2 = LR + 1               # live rows + appended bq column
    CH1 = _chunks(R, IC)
    CH2 = _chunks(cols2, IC)
    CHQ2 = _chunks(LR, IC)

    nc = bacc.Bacc(
        "TRN2", target_bir_lowering=False, debug=False, num_devices=NCORES
    )
    bf16d = mybir.dt.bfloat16

    # ---------------- I/O ----------------
    xf_d = nc.dram_tensor("xf", [C, N], bf16d, kind="ExternalInput")
    xc_d = nc.dram_tensor("xc", [C, R], f32r, kind="ExternalInput")
    mlive_d = nc.dram_tensor("mlive", [1, LR], f32, kind="ExternalInput")
    mbnd_d = nc.dram_tensor("mbnd", [1, max(BND, P)], f32,
                            kind="ExternalInput")
    # q/k weights pre-replicated 4x along the output dim (M=128)
    wqT1_d = nc.dram_tensor("wqT1", [C, P], f32r, kind="ExternalInput")
    wkT1_d = nc.dram_tensor("wkT1", [C, P], bf16d, kind="ExternalInput")
    wvT1_d = nc.dram_tensor("wvT1", [C, C], bf16d, kind="ExternalInput")
    wqT2_d = nc.dram_tensor("wqT2", [C, P], f32r, kind="ExternalInput")
    wkT2_d = nc.dram_tensor("wkT2", [C, P], bf16d, kind="ExternalInput")
    wvT2_d = nc.dram_tensor("wvT2", [C, C], bf16d, kind="ExternalInput")
    # consts columns: 0 sa_gamma, 1 gamma, 2/3 sa_gamma*sa_bv halves,
    # 6 sa_bq, 7 sa_bk, 8 bq, 9 bk (cols 6-9 replicated on all 4 groups)
    consts_d = nc.dram_tensor("consts", [P, 10], f32, kind="ExternalInput")
    out_d = nc.dram_tensor("outc", [C, R], f32, kind="ExternalOutput")

    groups = [[0, 1, 2, 3], [4, 5, 6, 7]]

    with tile.TileContext(nc) as tc:
        from contextlib import ExitStack

        ctx = ExitStack()
        with ctx:
            big = ctx.enter_context(tc.tile_pool(name="big", bufs=1))
            epool = ctx.enter_context(tc.tile_pool(name="epool", bufs=6))
            onpool = ctx.enter_context(tc.tile_pool(name="onpool", bufs=3))
            sqpool = ctx.enter_context(tc.tile_pool(name="sqpool", bufs=2))
            fbpool = ctx.enter_context(tc.tile_pool(name="fbpool", bufs=4))
            rcpool = ctx.enter_context(tc.tile_pool(name="rcpool", bufs=4))
            finpool = ctx.enter_context(tc.tile_pool(name="finpool", bufs=2))
            misc = ctx.enter_context(tc.tile_pool(name="misc", bufs=1))
            psA = ctx.enter_context(
                tc.tile_pool(name="psA", bufs=2, space="PSUM")
            )
            psS = ctx.enter_context(
                tc.tile_pool(name="psS", bufs=4, space="PSUM")
            )
            psO = ctx.enter_context(
                tc.tile_pool(name="psO", bufs=2, space="PSUM")
            )
            dram = ctx.enter_context(
                tc.tile_pool(name="dram", bufs=1, space="DRAM")
            )

            # ------------- persistent SBUF tiles -------------
            xc_sb = big.tile([P, 2, R], f32r, tag="xc", name="xc_sb")
            mlive_sb = big.tile([P, LR], f32, tag="mlive", name="mlive_sb")
            mbnd_sb = big.tile([P, max(BND, P)], f32, tag="mbnd",
                               name="mbnd_sb")
            xp_sb = big.tile([P, 2, R], f32, tag="xp", name="xp_sb")
            xp16_sb = big.tile([P, 2, KC], bf16, tag="xp16", name="xp16_sb")
            ff_sb = big.tile([P, 2, LR], f32r, tag="ff", name="ff_sb")
            wqT1_sb = big.tile([P, 2, P], f32r, tag="wqT1", name="wqT1_sb")
            wkT1_sb = big.tile([P, 2, P], bf16, tag="wkT1", name="wkT1_sb")
            wvT1_sb = big.tile([P, 2, C], bf16, tag="wvT1", name="wvT1_sb")
            wqT2_sb = big.tile([P, 2, P], f32r, tag="wqT2", name="wqT2_sb")
            wkT2_sb = big.tile([P, 2, P], bf16, tag="wkT2", name="wkT2_sb")
            wvT2_sb = big.tile([P, 2, C], bf16, tag="wvT2", name="wvT2_sb")
            consts_sb = big.tile([P, 10], f32, tag="consts", name="consts_sb")
            onesc_sb = big.tile([P, 1], bf16, tag="onesc", name="onesc_sb")
            onesr_sb = big.tile([1, P], f32r, tag="onesr", name="onesr_sb")
            bk16_sb = big.tile([D, 1], bf16, tag="bk16", name="bk16_sb")
            stats_sb = misc.tile([P, 8], f32, tag="stats", name="stats_sb")

            # ---- input DMAs: small/early tensors first so the layer-1
            # convs can start while the bulk of x streams in.
            nc.sync.dma_start(out=consts_sb[:], in_=consts_d[:])
            for k in range(2):
                cs = slice(k * P, (k + 1) * P)
                nc.sync.dma_start(out=wqT1_sb[:, k, :], in_=wqT1_d[cs, :])
                nc.sync.dma_start(out=wkT1_sb[:, k, :], in_=wkT1_d[cs, :])
                nc.sync.dma_start(out=wvT1_sb[:, k, :], in_=wvT1_d[cs, :])
            for h in range(2):
                hs = slice(h * IC, (h + 1) * IC)
                for k in range(2):
                    nc.sync.dma_start(
                        out=xc_sb[:, k, hs],
                        in_=xc_d[k * P : (k + 1) * P, hs],
                    )
            xf_sb = big.tile([P, 2, N], bf16, tag="xbig", name="xf_sb")
            for jc in range(4):
                js = slice(jc * (N // 4), (jc + 1) * (N // 4))
                for k in range(2):
                    nc.sync.dma_start(
                        out=xf_sb[:, k, js],
                        in_=xf_d[k * P : (k + 1) * P, js],
                    )
            for k in range(2):
                cs = slice(k * P, (k + 1) * P)
                nc.sync.dma_start(out=wqT2_sb[:, k, :], in_=wqT2_d[cs, :])
                nc.sync.dma_start(out=wkT2_sb[:, k, :], in_=wkT2_d[cs, :])
                nc.sync.dma_start(out=wvT2_sb[:, k, :], in_=wvT2_d[cs, :])
            nc.sync.dma_start(
                out=mlive_sb[:], in_=mlive_d[0, :].partition_broadcast(P)
            )
            nc.sync.dma_start(
                out=mbnd_sb[:], in_=mbnd_d[0, :].partition_broadcast(P)
            )
            nc.vector.memset(onesc_sb[:], 1.0)
            nc.vector.memset(onesr_sb[:].bitcast(f32), 1.0)
            nc.vector.tensor_copy(bk16_sb[:], consts_sb[0:D, 9:10])

            def conv_q(wT4_sb, bias_col, src_of, chlist, out_sb):
                """out (128 x width) = [wT tiled 4x].T @ src + bias: the
                D=32 query rows replicated on all four partition groups."""
                for off, w in chlist:
                    js = slice(off, off + w)
                    ps = psA.tile([P, IC], f32, tag="a", name="q_ps")
                    nc.tensor.matmul(
                        ps[:, 0:w], wT4_sb[:, 0, :], src_of(0, js),
                        start=True, stop=False,
                    )
                    nc.tensor.matmul(
                        ps[:, 0:w], wT4_sb[:, 1, :], src_of(1, js),
                        start=False, stop=True,
                    )
                    nc.vector.tensor_scalar_add(
                        out_sb[:, js], ps[:, 0:w],
                        consts_sb[:, bias_col : bias_col + 1],
                    )

            def conv_k(wT4_sb, bias_col, src0, src1, t0, ntl, k4_sb):
                """K conv for `ntl` consecutive key tiles starting at linear
                tile t0; tile t lands on partition group t%2, slot t//2."""
                ps = psA.tile([P, IC], f32, tag="a", name="k_ps")
                wct = ntl * P
                nc.tensor.matmul(
                    ps[:, 0:wct], wT4_sb[:, 0, :], src0,
                    start=True, stop=False,
                )
                nc.tensor.matmul(
                    ps[:, 0:wct], wT4_sb[:, 1, :], src1,
                    start=False, stop=True,
                )
                for i in range(ntl):
                    t = t0 + i
                    gs = slice(G * (t % 2), G * (t % 2) + G)
                    nc.vector.tensor_scalar_add(
                        k4_sb[gs, t // 2, :],
                        ps[gs, i * P : (i + 1) * P],
                        consts_sb[gs, bias_col : bias_col + 1],
                    )

            def conv_vT(wvT_sb, lhsT0, lhsT1, v_sb, t):
                """v_sb[:, t, :] = (src^T @ wvT) for key tile t (j on
                partitions, channels free)."""
                ps = psA.tile([P, C], f32, tag="a", name="v_ps")
                nc.tensor.matmul(
                    ps[:], lhsT0, wvT_sb[:, 0, :], start=True, stop=False
                )
                nc.tensor.matmul(
                    ps[:], lhsT1, wvT_sb[:, 1, :], start=False, stop=True
                )
                nc.scalar.copy(v_sb[:, t, :], ps[:])

            def attention(q_sb, k4_sb, v_sb, nt, chlist, den_fix, epilogue):
                """Row-chunk attention.  Per i-chunk: S^T = K-tile^T Q with
                score matmuls issued as 2-way row-tiled pairs (concurrent in
                the PE), E = exp(S^T), then O(c,i) accumulates with V^T
                slices stationary and E moving; the denominator row comes
                from an M=1 ones matmul, corrected by den_fix and
                replicated across partitions on GPSIMD."""
                LOOKP = 2
                npair = nt // 2
                for ich, (off, w) in enumerate(chlist):
                    is_ = slice(off, off + w)
                    accs = [
                        psO.tile([P, IC], f32, tag="o", name="acc")
                        for _ in range(2)
                    ]
                    den = psA.tile([1, IC], f32, tag="a", name="den")
                    es = {}

                    def s_exp(t, is_=is_, w=w):
                        g = t % 2
                        gs = slice(G * g, G * g + G)
                        sps = psS.tile([P, IC], f32, tag="s", name="s_ps")
                        nc.tensor.matmul(
                            sps[:, 0:w],
                            k4_sb[gs, t // 2, :],
                            q_sb[gs, is_],
                            start=True, stop=True,
                            tile_position=(G * g, 0),
                        )
                        e_sb = epool.tile([P, IC], bf16, tag="e", name="e_sb")
                        nc.scalar.activation(e_sb[:, 0:w], sps[:, 0:w], AF.Exp)
                        es[t] = e_sb

                    def s_pair(pp):
                        s_exp(2 * pp)
                        s_exp(2 * pp + 1)

                    for pp in range(min(LOOKP, npair)):
                        s_pair(pp)
                    for pp in range(npair):
                        if pp + LOOKP < npair:
                            s_pair(pp + LOOKP)
                        for t in (2 * pp, 2 * pp + 1):
                            e_sb = es.pop(t)
                            for ct in range(2):
                                nc.tensor.matmul(
                                    accs[ct][:, 0:w],
                                    v_sb[:, t, ct * P : (ct + 1) * P],
                                    e_sb[:, 0:w],
                                    start=(t == 0), stop=(t == nt - 1),
                                )
                            nc.tensor.matmul(
                                den[:, 0:w], onesc_sb[:], e_sb[:, 0:w],
                                start=(t == 0), stop=(t == nt - 1),
                            )
                    rrow = rcpool.tile([1, IC], f32, tag="rc", name="rrow")
                    den_fix(ich, off, w, den, rrow)
                    rrep = onpool.tile([P, IC], f32, tag="rr", name="rrep")
                    if USE_GPSIMD_BCAST:
                        nc.gpsimd.partition_broadcast(
                            rrep[:, 0:w], rrow[:, 0:w], channels=P
                        )
                    else:
                        rrow_r = rcpool.tile(
                            [1, IC], f32r, tag="rcr", name="rrow_r"
                        )
                        nc.vector.tensor_copy(rrow_r[:, 0:w], rrow[:, 0:w])
                        rrep_ps = psA.tile(
                            [P, IC], f32, tag="a", name="rrep_ps"
                        )
                        nc.tensor.matmul(
                            rrep_ps[:, 0:w], onesr_sb[:], rrow_r[:, 0:w],
                            start=True, stop=True,
                        )
                        nc.scalar.copy(rrep[:, 0:w], rrep_ps[:, 0:w])
                    epilogue(ich, off, w, accs, rrep)

            # ================= Layer 1: self-attention =================
            q1_sb = big.tile([P, R], bf16, tag="q", name="q1_sb")
            k1_sb = big.tile([P, NT // 2, P], bf16, tag="k", name="k1_sb")
            v1_sb = big.tile([P, NT, C], bf16, tag="v", name="v1_sb")

            conv_q(wqT1_sb, 6, lambda k, js: xc_sb[:, k, js], CH1, q1_sb)
            for jc in range(4):
                for h in range(2):
                    t0 = jc * 8 + h * 4
                    js = slice(t0 * P, (t0 + 4) * P)
                    conv_k(
                        wkT1_sb, 7,
                        xf_sb[:, 0, js], xf_sb[:, 1, js], t0, 4, k1_sb,
                    )
                for tsub in range(8):
                    t = jc * 8 + tsub
                    ts_ = slice(t * P, (t + 1) * P)
                    conv_vT(
                        wvT1_sb, xf_sb[:, 0, ts_], xf_sb[:, 1, ts_],
                        v1_sb, t,
                    )

            def den_fix1(ich, off, w, den, rrow):
                # ~18-bit reciprocal, 5x faster than full precision; den is
                # a positive softmax sum so the edge cases can't occur
                nc.vector.reciprocal_approx_fast(rrow[:, 0:w], den[:, 0:w])

            def epilogue1(ich, off, w, accs, rrep):
                io = slice(off, off + w)
                for ct in range(2):
                    # x' = sa_gamma * (O/den) + sa_gamma*bv + x, fused as
                    # ((O * sa_gamma) * rrep), then ((t + sgb) + x)
                    nc.vector.scalar_tensor_tensor(
                        xp_sb[:, ct, io], accs[ct][:, 0:w],
                        consts_sb[:, 0:1], rrep[:, 0:w],
                        op0=OP.mult, op1=OP.mult,
                    )
                    nc.vector.scalar_tensor_tensor(
                        xp_sb[:, ct, io], xp_sb[:, ct, io],
                        consts_sb[:, 2 + ct : 3 + ct],
                        xc_sb[:, ct, io].bitcast(f32),
                        op0=OP.add, op1=OP.add,
                    )
                    # bf16 cast only for the key-column prefix (AllGather)
                    lo, hi = max(off, 0), min(off + w, KC)
                    if lo < hi:
                        nc.scalar.copy(
                            xp16_sb[:, ct, lo:hi], xp_sb[:, ct, lo:hi]
                        )
                    # pure-bg rows: out = x' directly (ff = 0 there)
                    if off < PBT * P:
                        oe = min(off + w, PBT * P)
                        nc.sync.dma_start(
                            out=out_d[ct * P : (ct + 1) * P, off:oe],
                            in_=xp_sb[:, ct, off:oe],
                        )

            attention(q1_sb, k1_sb, v1_sb, NT, CH1, den_fix1, epilogue1)

            # ====== AllGather x' key columns within each batch group ======
            # Phase A: key cols [0, 512) (ready after i-chunk 0, overlaps
            # i-chunk 1).  Phase B (only if KPC == 5): cols [512, 640).
            # xbg keeps rank-major key-tile order [ct][rank][k][col]; the
            # scatter is one contiguous DMA per (rank, half, phase).
            xbg_sb = big.tile(
                [P, 2, RSH, KPC, P], bf16, tag="xbg", name="xbg_sb"
            )
            phases = [(0, min(KC, IC))]
            if KC > IC:
                phases.append((IC, KC - IC))
            for ph, (poff, pw) in enumerate(phases):
                ag_in = dram.tile(
                    [C, pw], bf16, tag=f"ag_in{ph}", name=f"ag_in{ph}"
                )
                ag_out = dram.tile(
                    [RSH, C, pw], bf16, tag=f"ag_out{ph}", name=f"ag_out{ph}"
                )
                for ct in range(2):
                    nc.sync.dma_start(
                        out=ag_in[ct * P : (ct + 1) * P, :],
                        in_=xp16_sb[:, ct, poff : poff + pw],
                    )
                nc.gpsimd.collective_compute(
                    "AllGather",
                    OP.bypass,
                    replica_groups=groups,
                    ins=[ag_in[:].opt()],
                    outs=[ag_out[:].opt()],
                )
                k0, nk = poff // P, pw // P
                for r in range(RSH):
                    for ct in range(2):
                        nc.sync.dma_start(
                            out=xbg_sb[:, ct, r, k0 : k0 + nk, :],
                            in_=ag_out[r, ct * P : (ct + 1) * P, :],
                        )

            # ============== Layer 2: masked cross-attention ==============
            # feature_f on live rows + its per-channel stats (cols 0-3)
            ffsq = misc.tile([P, LR], f32, tag="ffsq", name="ffsq")
            for ct in range(2):
                nc.vector.tensor_mul(
                    ff_sb[:, ct, :], mlive_sb[:],
                    xp_sb[:, ct, PBT * P : R].bitcast(f32),
                )
                nc.vector.tensor_reduce(
                    stats_sb[:, ct : ct + 1], ff_sb[:, ct, :].bitcast(f32),
                    axis=AX.X, op=OP.add,
                )
                nc.vector.tensor_mul(
                    ffsq[:],
                    ff_sb[:, ct, :].bitcast(f32),
                    ff_sb[:, ct, :].bitcast(f32),
                )
                nc.vector.tensor_reduce(
                    stats_sb[:, 2 + ct : 3 + ct], ffsq[:],
                    axis=AX.X, op=OP.add,
                )

            q2_sb = big.tile([P, cols2], bf16, tag="q2", name="q2_sb")
            conv_q(wqT2_sb, 8, lambda k, js: ff_sb[:, k, js], CHQ2, q2_sb)
            # appended bq column: the shared pure-background query row
            nc.vector.tensor_copy(q2_sb[:, LR : LR + 1], consts_sb[:, 8:9])

            # K2 / V2^T convs over the gathered key columns (rank-major
            # linear tile t = r*KPC + k).  Pure-bg tiles (k < PBT) use x'
            # directly (fb = x'); tiles k >= PBT get the (1-mask) multiply
            # (covers boundary/padding fg columns).
            k2_sb = big.tile([P, NT2 // 2, P], bf16, tag="k2", name="k2_sb")
            v2_sb = big.tile([P, NT2, C], bf16, tag="v2", name="v2_sb")
            # sub-chunks split at the pure/multiply boundary AND the
            # AllGather phase boundary (k=4); phase-A work for all ranks
            # runs first so phase B (KPC=5 only) never stalls the loop.
            subs_all = []
            for r in range(RSH):
                cuts = sorted({0, min(PBT, KPC), min(4, KPC), KPC})
                for k0, ke in zip(cuts, cuts[1:]):
                    subs_all.append((r, k0, ke - k0))
            subs_all.sort(key=lambda s: s[1] >= 4)
            for r, k0, nk in subs_all:
                if True:
                    t0 = r * KPC + k0
                    if k0 < PBT:
                        srcs = [xbg_sb[:, ct, r, k0 : k0 + nk, :]
                                for ct in range(2)]
                        tile_of = lambda ct, i, r=r, k0=k0: (
                            xbg_sb[:, ct, r, k0 + i, :]
                        )
                    else:
                        boff = (r * (KPC - PBT) + (k0 - PBT)) * P
                        mb3 = mbnd_sb[:, boff : boff + nk * P].rearrange(
                            "p (a b) -> p a b", a=nk
                        )
                        fbt = [
                            fbpool.tile([P, KPC - PBT, P], bf16, tag="fb",
                                        name=f"fb{ct}")
                            for ct in range(2)
                        ]
                        for ct in range(2):
                            nc.vector.tensor_mul(
                                fbt[ct][:, 0:nk, :], mb3,
                                xbg_sb[:, ct, r, k0 : k0 + nk, :],
                            )
                            nc.vector.tensor_sub(
                                fbt[ct][:, 0:nk, :],
                                xbg_sb[:, ct, r, k0 : k0 + nk, :],
                                fbt[ct][:, 0:nk, :],
                            )
                        srcs = [fbt[0][:, 0:nk, :], fbt[1][:, 0:nk, :]]
                        tile_of = lambda ct, i, fbt=fbt: fbt[ct][:, i, :]
                    conv_k(wkT2_sb, 9, srcs[0], srcs[1], t0, nk, k2_sb)
                    for i in range(nk):
                        conv_vT(
                            wvT2_sb, tile_of(0, i), tile_of(1, i),
                            v2_sb, t0 + i,
                        )

            def den_fix2(ich, off, w, den, rrow):
                # den += n_excl * exp(Q2_i . bk)  [excluded fg keys]
                qbk = psA.tile([1, IC], f32, tag="a", name="qbk")
                nc.tensor.matmul(
                    qbk[:, 0:w], bk16_sb[:], q2_sb[0:D, off : off + w],
                    start=True, stop=True,
                )
                eqbk = rcpool.tile([1, IC], f32, tag="eq", name="eqbk")
                nc.scalar.activation(eqbk[:, 0:w], qbk[:, 0:w], AF.Exp)
                dtot = rcpool.tile([1, IC], f32, tag="dt", name="dtot")
                nc.vector.scalar_tensor_tensor(
                    dtot[:, 0:w], eqbk[:, 0:w], float(n_excl), den[:, 0:w],
                    op0=OP.mult, op1=OP.add,
                )
                nc.vector.reciprocal_approx_fast(rrow[:, 0:w], dtot[:, 0:w])

            def epilogue2(ich, off, w, accs, rrep):
                # normalized sw_bg chunk; accumulate per-channel sum/sumsq
                # into stats cols 4-7; add the bq-column extra weight on
                # the chunk holding column LR.
                for ct in range(2):
                    onb = onpool.tile([P, IC], f32, tag="on", name="on2")
                    s1 = rcpool.tile([P, 1], f32, tag="s1", name="s1")
                    nc.vector.scalar_tensor_tensor(
                        onb[:, 0:w], accs[ct][:, 0:w], 1.0, rrep[:, 0:w],
                        op0=OP.mult, op1=OP.mult, accum_out=s1[:],
                    )
                    sqb = sqpool.tile([P, IC], f32, tag="sq", name="sq2")
                    s2 = rcpool.tile([P, 1], f32, tag="s2", name="s2")
                    # square+accumulate on the (otherwise idle) scalar
                    # engine so it runs parallel to the VE stats chain
                    nc.scalar.activation(
                        sqb[:, 0:w], onb[:, 0:w], AF.Square, accum_out=s2[:]
                    )
                    if ich == 0:
                        nc.vector.tensor_copy(
                            stats_sb[:, 4 + ct : 5 + ct], s1[:]
                        )
                        nc.vector.tensor_copy(
                            stats_sb[:, 6 + ct : 7 + ct], s2[:]
                        )
                    else:
                        nc.vector.tensor_add(
                            stats_sb[:, 4 + ct : 5 + ct],
                            stats_sb[:, 4 + ct : 5 + ct], s1[:],
                        )
                        nc.vector.tensor_add(
                            stats_sb[:, 6 + ct : 7 + ct],
                            stats_sb[:, 6 + ct : 7 + ct], s2[:],
                        )
                    if off <= LR < off + w:
                        bq = LR - off
                        nc.vector.scalar_tensor_tensor(
                            stats_sb[:, 4 + ct : 5 + ct],
                            onb[:, bq : bq + 1], float(BQW),
                            stats_sb[:, 4 + ct : 5 + ct],
                            op0=OP.mult, op1=OP.add,
                        )
                        nc.vector.scalar_tensor_tensor(
                            stats_sb[:, 6 + ct : 7 + ct],
                            sqb[:, bq : bq + 1], float(BQW),
                            stats_sb[:, 6 + ct : 7 + ct],
                            op0=OP.mult, op1=OP.add,
                        )

            attention(q2_sb, k2_sb, v2_sb, NT2, CH2, den_fix2, epilogue2)

            # ================== stats AllReduce + FMM ==================
            ar_in = dram.tile([P, 8], f32, tag="ar_in", name="ar_in")
            ar_out = dram.tile([P, 8], f32, tag="ar_out", name="ar_out")
            nc.sync.dma_start(out=ar_in[:], in_=stats_sb[:])
            nc.gpsimd.collective_compute(
                "AllReduce",
                OP.add,
                replica_groups=groups,
                ins=[ar_in[:].opt()],
                outs=[ar_out[:].opt()],
            )
            rst = misc.tile([P, 8], f32, tag="rst", name="rst")
            nc.sync.dma_start(out=rst[:], in_=ar_out[:])

            # var = (S2 - S1^2/N)/(N-1) + EPS (both channel-halves at once)
            varf = misc.tile([P, 2], f32, tag="varf", name="varf")
            varg = misc.tile([P, 2], f32, tag="varg", name="varg")
            ratio = misc.tile([P, 2], f32, tag="ratio", name="ratio")
            for var, s1s, s2s in ((varf, 0, 2), (varg, 4, 6)):
                nc.vector.tensor_mul(
                    var[:], rst[:, s1s : s1s + 2], rst[:, s1s : s1s + 2]
                )
                nc.vector.tensor_scalar(
                    var[:], var[:], -1.0 / N, None, op0=OP.mult
                )
                nc.vector.tensor_add(var[:], var[:], rst[:, s2s : s2s + 2])
                nc.vector.tensor_scalar(
                    var[:], var[:], 1.0 / (N - 1), EPS, op0=OP.mult, op1=OP.add
                )
            nc.vector.reciprocal(varf[:], varf[:])
            nc.vector.tensor_mul(varg[:], varg[:], varf[:])
            nc.scalar.activation(ratio[:], varg[:], AF.Sqrt)
            # fold in gamma
            nc.vector.tensor_scalar_mul(ratio[:], ratio[:], consts_sb[:, 1:2])

            # out = x' + (gamma * std_bg/std_f) * ff on live rows
            for ct in range(2):
                fin = finpool.tile([P, LR], f32, tag="fin", name="fin")
                nc.vector.scalar_tensor_tensor(
                    fin[:], ff_sb[:, ct, :].bitcast(f32),
                    ratio[:, ct : ct + 1], xp_sb[:, ct, PBT * P : R],
                    op0=OP.mult, op1=OP.add,
                )
                nc.sync.dma_start(
                    out=out_d[ct * P : (ct + 1) * P, PBT * P : R], in_=fin[:]
                )

    nc.compile()
    return nc


def _plan(mask):
    """Host-side permutation plan from the mask (per batch)."""
    mflat = [np.asarray(mask[b, 0], dtype=F32).ravel() for b in range(B)]
    perms, nbgs = [], []
    for mb in mflat:
        bg = np.flatnonzero(mb == 0.0)
        fg = np.flatnonzero(mb != 0.0)
        perms.append(np.concatenate([bg, fg]).astype(np.int64))
        nbgs.append(len(bg))
    NBGT_pad = 4 * (-(-max(-(-n // P) for n in nbgs) // 4))
    TL = 4 * ((min(n // P for n in nbgs)) // 4)
    return mflat, perms, nbgs, NBGT_pad, TL


def _prep_inputs(plan, x, mask, sa_wq, sa_bq, sa_wk, sa_bk, sa_wv, sa_bv,
                 sa_gamma, wq, bq, wk, bk, wv, bv, gamma):
    """Build the per-core input maps (host-side sharding + weight layout)."""
    mflat, perms, nbgs, NBGT_pad, TL = plan
    KPC = NBGT_pad // 4
    PBT = TL // 4
    LR = R - PBT * P
    BND = (KPC - PBT) * RSH * P
    x = np.ascontiguousarray(x, dtype=F32)

    import ml_dtypes

    BF16 = ml_dtypes.bfloat16
    wqT1 = np.ascontiguousarray(np.tile(sa_wq.T, (1, 4)), dtype=F32)
    wkT1 = np.ascontiguousarray(np.tile(sa_wk.T, (1, 4)).astype(BF16))
    wvT1 = np.ascontiguousarray(sa_wv.T.astype(BF16))
    wqT2 = np.ascontiguousarray(np.tile(wq.T, (1, 4)), dtype=F32)
    wkT2 = np.ascontiguousarray(np.tile(wk.T, (1, 4)).astype(BF16))
    wvT2 = np.ascontiguousarray(wv.T.astype(BF16))

    consts = np.zeros((P, 10), dtype=F32)
    consts[:, 0] = sa_gamma[0]
    consts[:, 1] = gamma[0]
    sgb = (sa_gamma[0] * sa_bv).astype(F32)
    consts[:, 2] = sgb[0:P]
    consts[:, 3] = sgb[P:C]
    for g in range(4):
        gs = slice(g * D, (g + 1) * D)
        consts[gs, 6] = sa_bq
        consts[gs, 7] = sa_bk
        consts[gs, 8] = bq
        consts[gs, 9] = bk

    in_maps = []
    for g in range(NCORES):
        b, r = g // RSH, g % RSH
        pb = perms[b]
        mb = mflat[b]
        xb = x[b].reshape(C, N)
        xfp = np.ascontiguousarray(xb[:, pb].astype(BF16))
        gtiles = list(range(r, NT, RSH))
        cols = np.concatenate(
            [pb[t * P : (t + 1) * P] for t in gtiles]
        )
        # boundary mask, rank-major: for rank rr, tiles k in [PBT, KPC)
        # correspond to global tiles 4k+rr
        mbnd = np.zeros((1, max(BND, P)), dtype=F32)
        pos = 0
        for rr in range(RSH):
            for k in range(PBT, KPC):
                gt = 4 * k + rr
                mbnd[0, pos : pos + P] = mb[pb[gt * P : (gt + 1) * P]]
                pos += P
        in_maps.append({
            "xf": xfp,
            "xc": np.ascontiguousarray(xb[:, cols]),
            "mlive": np.ascontiguousarray(
                mb[cols[PBT * P :]].reshape(1, LR)
            ),
            "mbnd": mbnd,
            "wqT1": wqT1, "wkT1": wkT1, "wvT1": wvT1,
            "wqT2": wqT2, "wkT2": wkT2, "wvT2": wvT2,
            "consts": consts,
        })
    return in_maps


def kernel(**inputs):
    from concourse import bass_utils

    plan = _plan(np.asarray(inputs["mask"]))
    _, perms, _, NBGT_pad, TL = plan
    key = (NBGT_pad, TL)
    if _CACHE.get("key") != key:
        _CACHE["nc"] = _build_bass(NBGT_pad, TL)
        _CACHE["key"] = key
    nc = _CACHE["nc"]

    in_maps = _prep_inputs(plan, **inputs)
    res = bass_utils.run_bass_kernel_spmd(
        nc, in_maps, core_ids=list(range(NCORES))
    )
    _CACHE["last_results"] = res

    out = np.empty((B, C, N), dtype=F32)
    for g in range(NCORES):
        b, r = g // RSH, g % RSH
        pb = perms[g // RSH]
        oc = res.results[g]["outc"]
        for k, t in enumerate(range(r, NT, RSH)):
            out[b][:, pb[t * P : (t + 1) * P]] = oc[:, k * P : (k + 1) * P]
    return out.reshape(B, C, HH, WW)
